# revision 31
# baseline (speedup 1.0000x reference)
# Transformer-XL style relative-position attention on 8 Trainium2 NeuronCores.
#
# Contract: kernel(**inputs) takes the FULL unsharded inputs and returns the
# FULL [8, 256, 1024] output. Internally shards data-parallel over batch:
# core b computes batch element b. No collectives needed.
#
# Math (per batch element):
#   cat = [h; x]                            [512, 1024]
#   q,k,v = split(cat @ Wqkv)               heads=16, dhead=64
#   RW    = R @ Wkr                         [1024, 1024] (relative pos keys)
#   dots  = (q+u) @ k^T + rel_shift((q+v) @ RW_h^T)
#   out   = softmax(dots*8^-1 + causal/mem band mask) @ v @ Wout
#
# Key design points:
#  * The combined mem/autoregressive mask keeps exactly the relative offsets
#    j - i in [0, 256]; in rel-coordinate s = j - i + 256 the valid window is
#    s in [256, 512] (257 values), so only 257 rows of RW are ever needed
#    (R rows 768..1023 and 0, since RW row (s+512)%1024 serves offset s).
#  * rel_shift is a per-row shear. SBUF cannot be addressed diagonally, but
#    DRAM can: write the [128, 258] valid band of BDs = (q+v) @ RWs^T to a
#    DRAM scratch laid out [128, 767] and read it back with the access
#    pattern [[766, 128], [1, 512]] (row stride 767-1) which realizes
#    band[i, j] = BDs[i, j - i + const]. The scratch is pre-filled with the
#    additive mask value -1e9 and the band write puts -1e9 in its pad
#    column, so the band read delivers band+mask in a single tensor.
#  * All matmul operands are fp16 (halves LDWEIGHTS streaming, which
#    dominates the PE pipe); accumulation stays fp32 in PSUM, and the
#    softmax/logit path (dots, exp, row sums) stays fp32.
#  * Weights are cast f32->f16 in-flight by gpsimd (SWDGE) cast-DMAs: no
#    compute-engine time and fully contiguous row reads.
#  * Normalization 1/S is applied per-partition to exp rows before the PE
#    transpose into the key-major layout used by the AV matmul.

import numpy as np

import concourse.bass as bass
import concourse.mybir as mybir
import concourse.tile as tile
from concourse import bacc, bass_utils
from concourse.masks import make_identity
from concourse.tile import add_dep_helper
from contextlib import ExitStack

F32 = mybir.dt.float32
F16 = mybir.dt.float16
AF = mybir.ActivationFunctionType

DIM = 1024
HEADS = 16
DHEAD = 64
B = 8
N = 256          # query tokens (x)
M = 256          # memory tokens (h)
T = M + N        # 512 keys
INNER = HEADS * DHEAD
SCALE = DHEAD ** -0.5
NEG = -30000.0   # fp16-representable; *0.125 still underflows exp
SW = 767         # BDs scratch width (relative offsets s = 1..767)
VAL0 = 255       # scratch col of first valid offset (s = 256)
NVALID = 257     # valid offsets s in [256, 512]
NV2 = 258        # band write width (one -1e9 pad col keeps mask intact)
WIN = 384        # per-query-block live key window (3 of 4 key tiles)
NBUF = 8         # BDs scratch buffering depth


def build_kernel():
    nc = bacc.Bacc("TRN2", target_bir_lowering=False, debug=False)

    x_d = nc.dram_tensor("x", [N, DIM], F32, kind="ExternalInput")
    h_d = nc.dram_tensor("h", [M, DIM], F32, kind="ExternalInput")
    wqkv_d = nc.dram_tensor("Wqkv", [DIM, 3 * INNER], F32, kind="ExternalInput")
    wkr_d = nc.dram_tensor("Wkr", [DIM, INNER], F32, kind="ExternalInput")
    r_d = nc.dram_tensor("R", [2 * T, DIM], F32, kind="ExternalInput")
    uu_d = nc.dram_tensor("uu", [128, 1], F32, kind="ExternalInput")
    vv_d = nc.dram_tensor("vv", [128, 1], F32, kind="ExternalInput")
    wout_d = nc.dram_tensor("Wout", [INNER, DIM], F32, kind="ExternalInput")
    out_d = nc.dram_tensor("out", [N, DIM], F32, kind="ExternalOutput")
    bds_d = nc.dram_tensor("bds_scratch", [NBUF, 128, SW], F16)
    neg_d = nc.dram_tensor("neg_seed", [128, 128], F16)

    with tile.TileContext(nc) as tc, ExitStack() as ctx:
        _body(ctx, tc, x_d, h_d, wqkv_d, wkr_d, r_d, uu_d, vv_d, wout_d,
              out_d, bds_d, neg_d)

    nc.compile()
    return nc


def _body(ctx, tc, x_d, h_d, wqkv_d, wkr_d, r_d, uu_d, vv_d, wout_d, out_d,
          bds_d, neg_d):
    nc = tc.nc

    const = ctx.enter_context(tc.tile_pool(name="const", bufs=1))
    persist = ctx.enter_context(tc.tile_pool(name="persist", bufs=1))
    ldpool = ctx.enter_context(tc.tile_pool(name="ld", bufs=4))
    wpool = ctx.enter_context(tc.tile_pool(name="wstream", bufs=3))
    work = ctx.enter_context(tc.tile_pool(name="work", bufs=4))
    ps_mid = ctx.enter_context(tc.tile_pool(name="ps_mid", bufs=5, space="PSUM"))
    ps_sml = ctx.enter_context(tc.tile_pool(name="ps_sml", bufs=3, space="PSUM"))

    # ---------------- constants ----------------
    ident = const.tile([128, 128], F32, tag="ident", name="ident")
    make_identity(nc, ident)
    ident_h = const.tile([128, 128], F16, tag="identh", name="ident_h")
    make_identity(nc, ident_h)

    # Scratch mask fill: every column outside the per-iteration band write
    # stays NEG; the band write puts NEG in its own pad column.
    neg_sb = const.tile([128, 128], F16, tag="zero", name="neg_sb")
    nc.gpsimd.memset(neg_sb, NEG)

    uu = const.tile([128, 1], F32, tag="uu", name="uu_sb")
    vv = const.tile([128, 1], F32, tag="vv", name="vv_sb")
    nc.sync.dma_start(out=uu, in_=uu_d[:, :])
    nc.sync.dma_start(out=vv, in_=vv_d[:, :])

    zinit = []
    for bi in range(NBUF):
        zi1 = nc.scalar.dma_start(out=bds_d[bi][:, 127:255], in_=neg_sb[:, 0:128])
        zi2 = nc.scalar.dma_start(out=bds_d[bi][:, 512:640], in_=neg_sb[:, 0:128])
        zinit.append((zi1, zi2))



    _cast_n = [0]

    def load16(dst, dram_slice):
        stg = ldpool.tile([128, DIM], F32, tag="stg", name=f"stg{_cast_n[0]}",
                          bufs=4)
        nc.sync.dma_start(out=stg, in_=dram_slice)
        if _cast_n[0] % 2 == 0:
            nc.vector.tensor_copy(dst, stg)
        else:
            nc.scalar.copy(dst, stg)
        _cast_n[0] += 1

    # ---------------- load + transpose x, h, R ----------------
    # cat token order: [h (0:256) | x (256:512)]; all loads are gpsimd
    # cast-DMAs (f32 -> f16 in flight). Order on the gpsimd queue matters:
    # activations and R first so compute can start, then weights grouped by
    # their consuming phase.
    cat16 = []
    for tt in range(4):
        t_ = ldpool.tile([128, DIM], F16, tag="xh", name=f"cat16_{tt}")
        src = h_d if tt < 2 else x_d
        load16(t_, src[(tt % 2) * 128:(tt % 2) * 128 + 128, :])
        cat16.append(t_)

    catT = [persist.tile([128, T], F16, tag=f"catT{dt}", name=f"catT{dt}")
            for dt in range(8)]
    for tt in range(4):
        for dt in range(8):
            tp = ps_sml.tile([128, 128], F16, tag="tp", name=f"tp_cat{tt}_{dt}")
            nc.tensor.transpose(tp, cat16[tt][:, dt * 128:(dt + 1) * 128],
                                ident_h)
            nc.vector.tensor_copy(catT[dt][:, tt * 128:(tt + 1) * 128], tp)

    # R rows needed: offsets s=256..511 -> rows 768..1023; s=512 -> row 0
    r16 = []
    for rt in range(2):
        t_ = ldpool.tile([128, DIM], F16, tag="rn", name=f"r16_{rt}", bufs=2)
        load16(t_, r_d[768 + rt * 128:768 + (rt + 1) * 128, :])
        r16.append(t_)
    r0 = const.tile([2, DIM], F32, tag="r0", name="r0_sb")
    nc.gpsimd.memset(r0, 0.0)
    nc.sync.dma_start(out=r0[0:1, :], in_=r_d[0:1, :])

    wq16 = [persist.tile([128, INNER], F16, tag=f"wq16_{dt}", name=f"wq16_{dt}")
            for dt in range(8)]
    for dt in range(8):
        load16(wq16[dt], wqkv_d[dt * 128:(dt + 1) * 128, 0:INNER])
    wkr16 = [persist.tile([128, INNER], F16, tag=f"wkr16_{dt}", name=f"wkr16_{dt}")
             for dt in range(8)]
    for dt in range(8):
        load16(wkr16[dt], wkr_d[dt * 128:(dt + 1) * 128, :])

    rsubT = [persist.tile([128, NV2], F16, tag=f"rsubT{dt}", name=f"rsubT{dt}")
             for dt in range(8)]
    for rt in range(2):
        for dt in range(8):
            tp = ps_sml.tile([128, 128], F16, tag="tp", name=f"tp_r{rt}_{dt}")
            nc.tensor.transpose(tp, r16[rt][:, dt * 128:(dt + 1) * 128],
                                ident_h)
            nc.scalar.copy(rsubT[dt][:, rt * 128:(rt + 1) * 128], tp)
    for dt in range(8):
        tp = ps_sml.tile([128, 2], F32, tag="tp", name=f"tp_r0_{dt}")
        nc.tensor.transpose(tp, r0[:, dt * 128:(dt + 1) * 128], ident[0:2, 0:2])
        nc.scalar.copy(rsubT[dt][:, 256:258], tp)

    # ---------------- projections ----------------
    # q_T (x tokens only) -> qu_T, qv_T [128 feat, 256 tok]
    quT = [persist.tile([128, N], F16, tag=f"quT{ft}", name=f"quT{ft}")
           for ft in range(8)]
    qvT = [persist.tile([128, N], F16, tag=f"qvT{ft}", name=f"qvT{ft}")
           for ft in range(8)]
    for ft in range(8):
        pq = ps_mid.tile([128, N], F32, tag="mid", name=f"ps_q{ft}")
        for dt in range(8):
            nc.tensor.matmul(pq, wq16[dt][:, ft * 128:(ft + 1) * 128],
                             catT[dt][:, M:T], start=(dt == 0), stop=(dt == 7))
        nc.vector.tensor_scalar_add(quT[ft], pq, uu)
        nc.vector.tensor_scalar_add(qvT[ft], pq, vv)

    # RWs_T[ft] = [128 feat, 258 offsets]
    rwsT = [persist.tile([128, NV2], F16, tag=f"rwsT{ft}", name=f"rwsT{ft}")
            for ft in range(8)]
    for ft in range(8):
        pr = ps_mid.tile([128, NV2], F32, tag="mid", name=f"ps_rw{ft}")
        for dt in range(8):
            nc.tensor.matmul(pr, wkr16[dt][:, ft * 128:(ft + 1) * 128],
                             rsubT[dt], start=(dt == 0), stop=(dt == 7))
        nc.scalar.copy(rwsT[ft], pr)

    # k_T[ft] = [128 feat, 512 tok]
    wk16 = [persist.tile([128, INNER], F16, tag=f"wk16_{dt}", name=f"wk16_{dt}")
            for dt in range(8)]
    for dt in range(8):
        load16(wk16[dt], wqkv_d[dt * 128:(dt + 1) * 128, INNER:2 * INNER])
    kT = [persist.tile([128, T], F16, tag=f"kT{ft}", name=f"kT{ft}")
          for ft in range(8)]
    for ft in range(8):
        pk = ps_mid.tile([128, T], F32, tag="mid", name=f"ps_k{ft}")
        for dt in range(8):
            nc.tensor.matmul(pk, wk16[dt][:, ft * 128:(ft + 1) * 128],
                             catT[dt], start=(dt == 0), stop=(dt == 7))
        nc.scalar.copy(kT[ft], pk)

    # val[tt] = [128 tok, 1024 feat]
    wv16 = [persist.tile([128, INNER], F16, tag=f"wv16_{dt}", name=f"wv16_{dt}")
            for dt in range(8)]
    for dt in range(8):
        load16(wv16[dt], wqkv_d[dt * 128:(dt + 1) * 128, 2 * INNER:3 * INNER])
    wo16 = [persist.tile([128, DIM], F16, tag=f"wo16_{dt}", name=f"wo16_{dt}")
            for dt in range(8)]
    for dt in range(8):
        load16(wo16[dt], wout_d[dt * 128:(dt + 1) * 128, :])
    val = [persist.tile([128, INNER], F16, tag=f"val{tt}", name=f"val{tt}")
           for tt in range(4)]
    for tt in range(4):
        pv = [ps_mid.tile([128, 512], F32, tag="mid", name=f"ps_v{tt}_{nh}")
              for nh in range(2)]
        for dt in range(8):
            lhs = catT[dt][:, tt * 128:(tt + 1) * 128]
            for nh in range(2):
                nc.tensor.matmul(pv[nh],
                                 lhs,
                                 wv16[dt][:, nh * 512:(nh + 1) * 512],
                                 start=(dt == 0), stop=(dt == 7))
        for nh in range(2):
            nc.scalar.copy(val[tt][:, nh * 512:(nh + 1) * 512], pv[nh])

    # ---------------- attention ----------------
    attn_outT = [persist.tile([128, N], F16, tag=f"aoT{ft}", name=f"aoT{ft}")
                 for ft in range(8)]

    last_read = [None] * NBUF
    it = 0
    for hh in range(HEADS):
        ft, ro = hh // 2, (hh % 2) * 64
        attnT = [work.tile([128, N], F16, tag="attnT", name=f"attnT{hh}_{jt}",
                           bufs=12) for jt in range(4)]
        nc.vector.memset(attnT[0][:, 128:256], 0.0)
        nc.vector.memset(attnT[3][:, 0:128], 0.0)
        for qb in range(2):
            bi = it % NBUF
            qsl = slice(qb * 128, (qb + 1) * 128)

            # BDs = (q+v) @ RWs^T  -> valid band (+ NEG pad col) to scratch
            pb = ps_mid.tile([128, NV2], F32, tag="mid", name=f"ps_b{it}")
            nc.tensor.matmul(pb, qvT[ft][ro:ro + 64, qsl],
                             rwsT[ft][ro:ro + 64, :], start=True, stop=True)
            bsb = work.tile([128, NV2], F16, tag="bsb", name=f"bsb{it}", bufs=6)
            nc.vector.tensor_copy(bsb[:, 0:NVALID], pb[:, 0:NVALID])
            nc.vector.memset(bsb[:, NVALID:NV2], NEG)
            w_inst = nc.sync.dma_start(
                out=bds_d[bi][:, VAL0:VAL0 + NV2], in_=bsb)
            for zi in zinit[bi]:
                add_dep_helper(w_inst.ins, zi.ins, sync=True,
                               reason="scratch WAW mask-init")
            if last_read[bi] is not None:
                add_dep_helper(w_inst.ins, last_read[bi].ins, sync=True,
                               reason="scratch WAR reuse")

            # A = (q+u) @ k^T over the live 384-key window
            pa = ps_mid.tile([128, WIN], F32, tag="mid", name=f"ps_a{it}")
            nc.tensor.matmul(pa, quT[ft][ro:ro + 64, qsl],
                             kT[ft][ro:ro + 64, qb * 128:qb * 128 + WIN],
                             start=True, stop=True)
            band_sb = work.tile([128, WIN], F16, tag="band", name=f"band{it}",
                                bufs=6)
            band = bass.AP(bds_d[bi].tensor,
                           bi * 128 * SW + VAL0,
                           [[SW - 1, 128], [1, WIN]])
            r_inst = nc.scalar.dma_start(out=band_sb, in_=band)
            add_dep_helper(r_inst.ins, w_inst.ins, sync=True,
                           reason="band RAW on scratch")
            for zi in zinit[bi]:
                add_dep_helper(r_inst.ins, zi.ins, sync=True,
                               reason="band RAW on mask-init")
            last_read[bi] = r_inst
            dots = work.tile([128, WIN], F32, tag="dots", name=f"dots{it}",
                             bufs=6)
            nc.vector.tensor_add(dots, pa, band_sb)

            # exp(+row sums), normalize into fp16 probabilities
            expt = work.tile([128, WIN], F16, tag="expt", name=f"expt{it}",
                             bufs=6)
            ssum = work.tile([128, 1], F32, tag="ssum", name=f"ssum{it}", bufs=6)
            nc.scalar.activation(expt, dots, AF.Exp, bias=0.0, scale=SCALE,
                                 accum_out=ssum)
            rcp = work.tile([128, 1], F32, tag="rcp", name=f"rcp{it}", bufs=6)
            nc.vector.reciprocal(rcp, ssum)
            expn = work.tile([128, WIN], F16, tag="expn", name=f"expn{it}",
                             bufs=6)
            nc.vector.tensor_scalar_mul(expn, expt, rcp)

            # transpose attn rows into key-major attnT tiles (3 live tiles)
            for w in range(3):
                jt = qb + w
                tp = ps_sml.tile([128, 128], F16, tag="tp", name=f"tp_e{it}_{w}")
                nc.tensor.transpose(tp, expn[:, w * 128:(w + 1) * 128], ident_h)
                nc.vector.tensor_copy(attnT[jt][:, qsl], tp)
            it += 1

        # out_T[h] = [64 d, 256 i]
        pav = ps_sml.tile([64, N], F32, tag="tp", name=f"ps_av{hh}")
        for jt in range(4):
            nc.tensor.matmul(pav, val[jt][:, hh * 64:hh * 64 + 64],
                             attnT[jt], start=(jt == 0), stop=(jt == 3))
        nc.scalar.copy(attn_outT[ft][ro:ro + 64, :], pav)

    # ---------------- output projection ----------------
    for tt in range(2):
        pp = [ps_mid.tile([128, 512], F32, tag="mid", name=f"ps_o{tt}_{nh}")
              for nh in range(2)]
        for itile in range(8):
            lhs = attn_outT[itile][:, tt * 128:(tt + 1) * 128]
            for nh in range(2):
                nc.tensor.matmul(pp[nh],
                                 lhs, wo16[itile][:, nh * 512:(nh + 1) * 512],
                                 start=(itile == 0), stop=(itile == 7))
        osb = work.tile([128, DIM], F32, tag="osb", name=f"osb{tt}", bufs=2)
        for nh in range(2):
            nc.scalar.copy(osb[:, nh * 512:(nh + 1) * 512], pp[nh])
        nc.sync.dma_start(out=out_d[tt * 128:(tt + 1) * 128, :], in_=osb)


_NC_CACHE = {}


def _get_nc():
    if "nc" not in _NC_CACHE:
        _NC_CACHE["nc"] = build_kernel()
    return _NC_CACHE["nc"]


def _run(inputs, trace=False):
    x = np.ascontiguousarray(np.asarray(inputs["x"], dtype=np.float32))
    h = np.ascontiguousarray(np.asarray(inputs["h"], dtype=np.float32))
    wqkv = np.ascontiguousarray(np.asarray(inputs["Wqkv"], dtype=np.float32))
    wkr = np.ascontiguousarray(np.asarray(inputs["Wkr"], dtype=np.float32))
    r = np.ascontiguousarray(np.asarray(inputs["R"], dtype=np.float32))
    u = np.asarray(inputs["u"], dtype=np.float32)
    v = np.asarray(inputs["v"], dtype=np.float32)
    wout = np.ascontiguousarray(np.asarray(inputs["Wout"], dtype=np.float32))
    uu = np.ascontiguousarray(np.tile(u, 2).reshape(128, 1))
    vv = np.ascontiguousarray(np.tile(v, 2).reshape(128, 1))

    nc = _get_nc()
    in_maps = [
        {"x": x[b], "h": h[b], "Wqkv": wqkv, "Wkr": wkr, "R": r,
         "uu": uu, "vv": vv, "Wout": wout}
        for b in range(B)
    ]
    res = bass_utils.run_bass_kernel_spmd(
        nc, in_maps, core_ids=list(range(B)), trace=trace)
    out = np.stack([res.results[b]["out"] for b in range(B)])
    return out.astype(np.float32), res


def kernel(**inputs):
    out, _ = _run(inputs, trace=False)
    return out
